# revision 3
# baseline (speedup 1.0000x reference)
import numpy as np
import jax
import jax.numpy as jnp

# Hardcoded problem shapes (nn_BFMFB_71296457113746)
B, DIM, H, W = 8, 128, 64, 64
PAIR = 2 * DIM
D_STATE, D_CONV, EXPAND = 64, 4, 2
D_INNER = EXPAND * PAIR
HEADDIM = 64
NHEADS = D_INNER // HEADDIM
CONV_DIM = D_INNER + 2 * D_STATE
D_IN_PROJ = 2 * D_INNER + 2 * D_STATE + NHEADS
BANDS = ((0.0, 0.25), (0.25, 0.5), (0.5, 1.0))
EPS = 1e-5


def _dft_consts():
    # Forward ortho DFT with fftshift folded into the rows; inverse ortho DFT
    # with ifftshift folded into the columns. n=64 (even) so shift = roll 32.
    n = H
    j = np.arange(n)
    ang = -2.0 * np.pi * np.outer(j, j) / n
    Fr = np.cos(ang) / np.sqrt(n)
    Fi = np.sin(ang) / np.sqrt(n)
    Fsr = np.roll(Fr, 32, axis=0).astype(np.float32)
    Fsi = np.roll(Fi, 32, axis=0).astype(np.float32)
    # inverse = conj(F) for ortho; ifftshift folded on input index (columns)
    Gr = np.roll(Fr, 32, axis=1).astype(np.float32)
    Gi = np.roll(-Fi, 32, axis=1).astype(np.float32)
    return Fsr, Fsi, Gr, Gi


FSR, FSI, GR, GI = _dft_consts()


def _band_masks_np():
    fy = (np.arange(H) - H // 2) / H
    fx = (np.arange(W) - W // 2) / W
    r = np.sqrt(fy[:, None] ** 2 + fx[None, :] ** 2) / (0.5 * np.sqrt(2.0))
    out = []
    for i, (lo, hi) in enumerate(BANDS):
        m = (r >= lo) & ((r <= hi) if i == len(BANDS) - 1 else (r < hi))
        out.append(m.astype(np.float32))
    return np.stack(out)


MASKS = _band_masks_np()


def _sigmoid(x):
    return 0.5 * (1.0 + jnp.tanh(0.5 * x))


def _silu(x):
    return x * _sigmoid(x)


def _gelu(x):
    e = jax.scipy.special.erf(x * np.float32(0.7071067811865476))
    return 0.5 * x * e + 0.5 * x


def _softplus(x):
    # ln(1+e^x), written to avoid the compiler's Softplus pattern-match
    return jnp.log(0.5 + 0.5 * jnp.exp(x)) + np.float32(0.6931471805599453)


def _rsqrt(x):
    return jnp.exp(-0.5 * jnp.log(x))


def conv1x1(x, Wm):
    # [C,H,W] x [O,C] -> [O,H,W]
    return jnp.einsum('chw,oc->ohw', x, Wm)


def bn2d(x, p):
    s = p["w"] * _rsqrt(p["rv"] + EPS)
    return x * s[:, None, None] + (p["b"] - p["rm"] * s)[:, None, None]


def mamba2_ssd(tokens, p, ln_w, ln_b):
    # tokens [N,L,PAIR]; ln over last dim, then Mamba2 via quadratic SSD form.
    N, L, _ = tokens.shape
    mu = tokens.mean(-1, keepdims=True)
    var = jnp.mean((tokens - mu) ** 2, -1, keepdims=True)
    t = (tokens - mu) * _rsqrt(var + EPS) * ln_w + ln_b

    zxbcdt = t @ p["Win"].T
    z = zxbcdt[..., :D_INNER]
    xBC = zxbcdt[..., D_INNER:D_INNER + CONV_DIM]
    dt = zxbcdt[..., D_INNER + CONV_DIM:]

    # depthwise causal conv over L (cross-correlation, left zero-pad)
    xpad = jnp.pad(xBC, ((0, 0), (D_CONV - 1, 0), (0, 0)))
    conv = sum(xpad[:, k:k + L, :] * p["conv_w"][:, k] for k in range(D_CONV))
    xBC = _silu(conv + p["conv_b"])
    xh = xBC[..., :D_INNER]
    Bm = xBC[..., D_INNER:D_INNER + D_STATE]
    Cm = xBC[..., D_INNER + D_STATE:]

    dt = _softplus(dt + p["dt_bias"])                    # [N,L,Hh]
    A = jnp.exp(p["A_log"])
    tril = np.tril(np.ones((L, L), dtype=np.float32))
    S = jnp.einsum('ts,nsh->nth', tril, -A[None, None, :] * dt)  # [N,L,Hh]
    seg = S[:, :, None, :] - S[:, None, :, :]                  # [N,t,s,Hh]
    trif = tril[None, :, :, None]
    seg = seg * trif + (trif - 1.0) * 10000.0
    M = jnp.exp(seg)                                           # [N,t,s,Hh]
    G = jnp.einsum('ntk,nsk->nts', Cm, Bm)
    Wt = M * G[..., None]                                      # [N,t,s,Hh]

    xh4 = xh.reshape(N, L, NHEADS, HEADDIM)
    dtx = dt[..., None] * xh4                                  # [N,L,Hh,P]
    ys = jnp.einsum('ntsh,nshp->nthp', Wt, dtx)
    y = ys + p["D"][None, None, :, None] * xh4
    yg = y.reshape(N, L, D_INNER) * _silu(z)
    yg = yg * _rsqrt(jnp.mean(yg * yg, -1, keepdims=True) + EPS) * p["norm_w"]
    return yg @ p["Wout"].T


def forward_single(x, params, masks):
    # x [DIM,H,W] — one batch element per core
    shortcut = x
    mu = x.mean(0, keepdims=True)
    var = jnp.mean((x - mu) ** 2, 0, keepdims=True)
    xn = (x - mu) * _rsqrt(var + EPS) * params["pre_w"][:, None, None] \
        + params["pre_b"][:, None, None]

    # FFT2 (ortho) + fftshift, as real matmuls
    t1r = jnp.einsum('chw,xw->chx', xn, FSR)
    t1i = jnp.einsum('chw,xw->chx', xn, FSI)
    Zr = jnp.einsum('yh,chx->cyx', FSR, t1r) - jnp.einsum('yh,chx->cyx', FSI, t1i)
    Zi = jnp.einsum('yh,chx->cyx', FSR, t1i) + jnp.einsum('yh,chx->cyx', FSI, t1r)
    xp = jnp.concatenate([Zr, Zi], axis=0)                     # [PAIR,H,W]

    acc = jnp.zeros_like(xp)
    for i in range(3):
        bp = params["bands"][i]
        m = masks[i]
        xb = _gelu(bn2d(conv1x1(xp * m[None], bp["pre_W"]), bp["pre_bn"]))
        # row branch: sequences along W
        t = xb.transpose(1, 2, 0)                              # [H,W,PAIR]
        yr = mamba2_ssd(t, params["row_mamba"], bp["row_w"], bp["row_b"])
        yr = yr.transpose(2, 0, 1)                             # [PAIR,H,W]
        # col branch: sequences along H
        t = xb.transpose(2, 1, 0)                              # [W,H,PAIR]
        yc = mamba2_ssd(t, params["col_mamba"], bp["col_w"], bp["col_b"])
        yc = yc.transpose(2, 1, 0)                             # [PAIR,H,W]
        g = _sigmoid(conv1x1(jnp.concatenate([yr, yc], 0), bp["gate_W"])
                           + bp["gate_b"][:, None, None])
        yb = bn2d(conv1x1(g * yr + (1.0 - g) * yc, bp["post_W"]), bp["post_bn"])
        acc = acc + yb * m[None]

    accR, accI = acc[:DIM], acc[DIM:]
    # inverse: xs = G_H @ (accR + i accI) @ G_W^T  (ifftshift folded in G cols)
    t2r = jnp.einsum('chw,xw->chx', accR, GR) - jnp.einsum('chw,xw->chx', accI, GI)
    t2i = jnp.einsum('chw,xw->chx', accR, GI) + jnp.einsum('chw,xw->chx', accI, GR)
    xsr = jnp.einsum('yh,chx->cyx', GR, t2r) - jnp.einsum('yh,chx->cyx', GI, t2i)
    xsi = jnp.einsum('yh,chx->cyx', GR, t2i) + jnp.einsum('yh,chx->cyx', GI, t2r)
    sp = jnp.concatenate([xsr, xsi], axis=0)                   # [PAIR,H,W]

    out = _gelu(bn2d(conv1x1(sp, params["sp_W"]), params["sp_bn"]))
    return shortcut + params["gamma"][0] * out


_fwd = None


def kernel(x, params):
    global _fwd
    x = np.asarray(x, dtype=np.float32)
    params_np = jax.tree_util.tree_map(lambda a: np.asarray(a, dtype=np.float32), params)
    if _fwd is None:
        _fwd = jax.pmap(forward_single, in_axes=(0, None, None))
    out = _fwd(x, params_np, MASKS)
    return np.asarray(out, dtype=np.float32)
